# revision 14
# baseline (speedup 1.0000x reference)
"""Trainium2 Bass kernel for a GNN message-passing layer.

Strategy (window-balanced node sharding, host-side gather, no collectives):
  - Destination nodes are grouped into 784 windows of 128. Windows are
    sorted by edge-tile count and dealt round-robin to the 8 cores, so one
    NEFF (per-iteration tile counts = the max of each deal-group) fits all
    cores with ~1% padding.
  - Host pre-gathers x[src], x[dst] into an edge-major bf16 stream
    xsd[128, e_pad] (rows 0:64 = x[src]^T, 64:128 = x[dst]^T) plus
    attrA[17, e_pad] (edge_attr^T with a ones row for the b1 fold).
  - Device, per <=512-edge chunk: h^T via 2 wide matmuls (K=128 xsd,
    K=17 attr; b1 folded into attr weights; an extra output column makes
    silu emit a constant-1 row so b2 folds into the msg matmul). Per
    128-edge tile: msg edge-major (lhsT = h-slice), then node-major
    scatter-add agg[128n, 64f] += sel^T @ msg in PSUM (sel built one
    chunk per DVE is_equal via stride-0 broadcast AP).
  - Window tail: agg transposed via the PE (identity matmul) into the
    feat-major node-MLP input; out^T = silu(W3^T [x_win; agg] + b3) with
    b3 on the activation bias port; host transposes/permutes back.

All matmuls bf16 (f32 PSUM accumulate).
"""

import numpy as np
import ml_dtypes

P = 128
H = 64
ED = 16
N_CORES = 8
CHUNK = 4          # max tiles per chunk (4*128 = 512 edges, one PSUM bank)


def _chunk_sizes(t_w):
    nch = (t_w + CHUNK - 1) // CHUNK
    q, r = divmod(t_w, nch)
    return [q + 1] * r + [q] * (nch - r)


# ---------------------------------------------------------------- host prep

def _silu_inv_one():
    """z with z*sigmoid(z) == 1 (float64 Newton)."""
    z = 1.3
    for _ in range(50):
        s = 1.0 / (1.0 + np.exp(-z))
        f = z * s - 1.0
        df = s * (1.0 + z * (1.0 - s))
        z -= f / df
    return z


def _prep(x, edge_index, edge_attr):
    """Sort/pad edges into the balanced per-core layout; host-gather x."""
    bf16 = ml_dtypes.bfloat16
    n_nodes = x.shape[0]
    nwg = (n_nodes + P - 1) // P                       # global windows
    nwg_pad = ((nwg + N_CORES - 1) // N_CORES) * N_CORES
    nw = nwg_pad // N_CORES                            # iterations per core
    npc_pad = nw * P

    src = edge_index[0].astype(np.int64)
    dst = edge_index[1].astype(np.int64)
    e = src.shape[0]

    gw = dst // P                                      # global window
    dloc = dst - gw * P

    cntw = np.bincount(gw, minlength=nwg_pad)
    t_g = np.maximum((cntw + P - 1) // P, 1)           # tiles per window

    order_w = np.argsort(-t_g, kind="stable")          # rank -> window
    rank_of = np.empty(nwg_pad, dtype=np.int64)
    rank_of[order_w] = np.arange(nwg_pad)

    # per-iteration tile count = tiles of the largest window in the deal
    tw = t_g[order_w[np.arange(nw) * N_CORES]].astype(np.int64)
    sw = tw * P
    base = np.concatenate([[0], np.cumsum(sw)[:-1]])
    e_pad = int(sw.sum())
    t_tot = int(tw.sum())

    core = rank_of[gw] % N_CORES
    wslot = rank_of[gw] // N_CORES
    key = core * nw + wslot
    order = np.argsort(key, kind="stable")
    key_s = key[order]
    counts = np.bincount(key_s, minlength=N_CORES * nw)
    starts = np.concatenate([[0], np.cumsum(counts)[:-1]])
    rank = np.arange(e, dtype=np.int64) - starts[key_s]
    core_s = key_s // nw
    slot = base[key_s - core_s * nw] + rank

    src_s = src[order]
    dst_s = dst[order]
    dloc_s = dloc[order]

    # edge-major streams (pad cols stay 0 / dloc -1)
    xb = x.astype(bf16)
    xsA = np.zeros((N_CORES, H + ED + 1, e_pad), dtype=bf16)
    xsA[:, H + ED, :] = bf16(1.0)
    xsA[core_s, :, slot] = np.concatenate(
        [xb[src_s], edge_attr[order].astype(bf16),
         np.ones((e, 1), dtype=bf16)], axis=1)

    xsB = np.zeros((N_CORES, H, e_pad), dtype=bf16)
    xsB[core_s, :, slot] = xb[dst_s]

    dloc_slots = np.full((N_CORES, e_pad), -1.0, dtype=np.float32)
    dloc_slots[core_s, slot] = dloc_s.astype(np.float32)
    dstc = np.ascontiguousarray(
        dloc_slots.reshape(N_CORES, t_tot, P).transpose(0, 2, 1)
    ).astype(bf16)                                     # [C, 128, Ttot]

    # node features per (core, iteration window)
    xTpad = np.zeros((H, nwg_pad * P), dtype=bf16)
    xTpad[:, :n_nodes] = x.T.astype(bf16)
    xTn = np.empty((N_CORES, H, npc_pad), dtype=bf16)
    colidx = np.arange(P)
    for c in range(N_CORES):
        gws = order_w[np.arange(nw) * N_CORES + c]
        idx = (gws[:, None] * P + colidx[None, :]).ravel()
        xTn[c] = xTpad[:, idx]

    struct = {"nw": nw, "n_nodes": n_nodes, "nwg_pad": nwg_pad,
              "npc_pad": npc_pad, "e_pad": e_pad, "t_tot": t_tot,
              "tw": tw, "order_w": order_w}
    arrays = {"xsA": xsA, "xsB": xsB, "dstc": dstc, "xTn": xTn}
    return struct, arrays


def _prep_consts(W1, b1, W2, b2, W3, b3):
    bf16 = ml_dtypes.bfloat16
    z1 = _silu_inv_one()

    w1sa = np.zeros((H + ED + 1, H + 1), dtype=bf16)
    w1sa[0:H, :H] = W1[0:H, :].astype(bf16)               # W1a (src)
    w1sa[H:H + ED, :H] = W1[2 * H:2 * H + ED, :].astype(bf16)  # W1c (attr)
    w1sa[H + ED, :H] = b1.astype(bf16)
    w1sa[H + ED, H] = bf16(z1)      # silu -> constant 1.0 row

    w1db = np.zeros((H, H + 1), dtype=bf16)
    w1db[:, :H] = W1[H:2 * H, :].astype(bf16)             # W1b (dst)

    w2a = np.zeros((H + 1, H), dtype=bf16)
    w2a[0:H, :] = W2.astype(bf16)
    w2a[H, :] = b2.astype(bf16)

    iorat4 = np.broadcast_to(
        np.tile(np.arange(P, dtype=np.float32), CHUNK), (P, CHUNK * P)
    ).copy().astype(bf16)

    consts = {
        "w1sa": w1sa,
        "w1db": w1db,
        "w2a": w2a,
        "w3a": W3[0:H, :].astype(bf16),
        "w3b": W3[H:2 * H, :].astype(bf16),
        "b3c": b3.reshape(H, 1).astype(np.float32),
        "iorat4": iorat4,
        "ident": np.eye(P, dtype=bf16),
    }
    return consts


# ---------------------------------------------------------------- device IR

def _build(struct):
    import concourse.mybir as mybir
    import concourse.tile as tile
    from concourse import bacc

    nw = struct["nw"]
    npc_pad = struct["npc_pad"]
    e_pad = struct["e_pad"]
    t_tot = struct["t_tot"]
    tw = struct["tw"]

    bf = mybir.dt.bfloat16
    f32 = mybir.dt.float32
    AF = mybir.ActivationFunctionType
    ALU = mybir.AluOpType

    nc = bacc.Bacc("TRN2", target_bir_lowering=False)

    xsA = nc.dram_tensor("xsA", [H + ED + 1, e_pad], bf, kind="ExternalInput")
    xsB = nc.dram_tensor("xsB", [H, e_pad], bf, kind="ExternalInput")
    dstc = nc.dram_tensor("dstc", [P, t_tot], bf, kind="ExternalInput")
    xTn = nc.dram_tensor("xTn", [H, npc_pad], bf, kind="ExternalInput")
    w1sa = nc.dram_tensor("w1sa", [H + ED + 1, H + 1], bf,
                          kind="ExternalInput")
    w1db = nc.dram_tensor("w1db", [H, H + 1], bf, kind="ExternalInput")
    w2a = nc.dram_tensor("w2a", [H + 1, H], bf, kind="ExternalInput")
    w3a = nc.dram_tensor("w3a", [H, H], bf, kind="ExternalInput")
    w3b = nc.dram_tensor("w3b", [H, H], bf, kind="ExternalInput")
    b3c = nc.dram_tensor("b3c", [H, 1], f32, kind="ExternalInput")
    iorat4 = nc.dram_tensor("iorat4", [P, CHUNK * P], bf, kind="ExternalInput")
    ident = nc.dram_tensor("ident", [P, P], bf, kind="ExternalInput")
    outT = nc.dram_tensor("outT", [H, npc_pad], f32, kind="ExternalOutput")

    with tile.TileContext(nc) as tc:
        with (
            tc.tile_pool(name="const", bufs=1) as cp,
            tc.tile_pool(name="win", bufs=2) as wp,
            tc.tile_pool(name="work", bufs=3) as kp,
            tc.tile_pool(name="nodein", bufs=2) as np_,
            tc.tile_pool(name="outp", bufs=2) as op_,
            tc.tile_pool(name="ps_h", bufs=2, space="PSUM") as ph,
            tc.tile_pool(name="ps_m", bufs=2, space="PSUM") as pm,
            tc.tile_pool(name="ps_a", bufs=2, space="PSUM") as pa,
            tc.tile_pool(name="ps_x", bufs=1, space="PSUM") as px,
        ):
            def load_const(t, shape, dt):
                s = cp.tile(shape, dt, tag=t.name)
                nc.sync.dma_start(out=s[:], in_=t[:])
                return s

            w1sat = load_const(w1sa, [H + ED + 1, H + 1], bf)
            w1dbt = load_const(w1db, [H, H + 1], bf)
            w2at = load_const(w2a, [H + 1, H], bf)
            w3at = load_const(w3a, [H, H], bf)
            w3bt = load_const(w3b, [H, H], bf)
            b3t = load_const(b3c, [H, 1], f32)
            iot = load_const(iorat4, [P, CHUNK * P], bf)
            identt = load_const(ident, [P, P], bf)

            # flat chunk schedule: (w, c0, tpc, first, last)
            base = np.concatenate([[0], np.cumsum(tw * P)[:-1]]).astype(int)
            baseT = np.concatenate([[0], np.cumsum(tw)[:-1]]).astype(int)
            chunks = []
            for w in range(nw):
                t_w = int(tw[w])
                c0 = 0
                for sz in _chunk_sizes(t_w):
                    chunks.append((w, c0, sz, c0 == 0, c0 + sz == t_w))
                    c0 += sz

            wtiles = {}   # per-window SBUF tiles
            wpsum = {}    # per-window agg PSUM
            ctiles = {}   # per-chunk tiles
            gtiles = {}   # per-group (8-window) SBUF tiles

            GRP = 8
            ngrp = (nw + GRP - 1) // GRP
            gw0 = [g * GRP for g in range(ngrp)]
            gw1 = [min((g + 1) * GRP, nw) for g in range(ngrp)]

            def emit_group_dma(g):
                w0, w1 = gw0[g], gw1[g]
                colT = int(baseT[w0])
                tg = int(sum(int(tw[w]) for w in range(w0, w1)))
                dct8 = np_.tile([P, tg], bf, tag="dct8")
                nc.sync.dma_start(out=dct8[:], in_=dstc[:, colT:colT + tg])
                nit8 = np_.tile([H, GRP * P], bf, tag="nit8")
                nc.sync.dma_start(out=nit8[:, :(w1 - w0) * P],
                                  in_=xTn[:, w0 * P:w1 * P])
                oo8 = op_.tile([H, GRP * P], f32, tag="oo8")
                gtiles[g] = (dct8, nit8, oo8, colT)

            def emit_dma(w):
                t_w = int(tw[w])
                s_w = t_w * P
                col = int(base[w])
                tA = wp.tile([H + ED + 1, s_w], bf, tag="tA")
                nc.sync.dma_start(out=tA[:], in_=xsA[:, col:col + s_w])
                tB = wp.tile([H, s_w], bf, tag="tB")
                nc.sync.dma_start(out=tB[:], in_=xsB[:, col:col + s_w])
                wtiles[w] = (tA, tB)

            def emit_h1(k):
                w, c0, tpc, first, _ = chunks[k]
                tA, tB = wtiles[w]
                if first:
                    wpsum[w] = pa.tile([P, H], f32, tag="agg", name="aggps")
                cw = tpc * P
                cols = slice(c0 * P, c0 * P + cw)
                hps = ph.tile([H + 1, CHUNK * P], f32, tag="hps")
                nc.tensor.matmul(hps[:, :cw], lhsT=w1sat[:],
                                 rhs=tA[:, cols],
                                 start=True, stop=False,
                                 skip_group_check=True)
                ctiles[k] = hps

            def emit_h(k):
                w, c0, tpc, first, _ = chunks[k]
                tA, tB = wtiles[w]
                g = w // GRP
                dct8 = gtiles[g][0]
                doff = int(baseT[w]) - gtiles[g][3]
                cw = tpc * P
                cols = slice(c0 * P, c0 * P + cw)
                hps = ctiles.pop(k)
                nc.tensor.matmul(hps[:, :cw], lhsT=w1dbt[:],
                                 rhs=tB[:, cols],
                                 start=False, stop=True,
                                 skip_group_check=True)
                hsb = kp.tile([H + 1, CHUNK * P], bf, tag="hsb")
                nc.scalar.activation(hsb[:, :cw], hps[:, :cw], AF.Silu)
                # one-hot sel for the whole chunk:
                # sel[p, t, n] = (dloc[tile t, edge p] == n)
                selc = kp.tile([P, CHUNK * P], bf, tag="selc")
                nc.vector.tensor_tensor(
                    out=selc[:, :cw].rearrange("p (c o) -> p c o", o=P),
                    in0=dct8[:, doff + c0:doff + c0 + tpc]
                        .rearrange("p (c o) -> p c o", o=1)
                        .to_broadcast([P, tpc, P]),
                    in1=iot[:, :cw].rearrange("p (c o) -> p c o", o=P),
                    op=ALU.is_equal,
                )
                ctiles[k] = (hsb, selc)

            def emit_msg(k):
                _, _, tpc, _, _ = chunks[k]
                hsb, _ = ctiles[k]
                msgps = pm.tile([P, CHUNK * H], f32, tag="msgps")
                for t in range(tpc):
                    nc.tensor.matmul(
                        msgps[:, t * H:(t + 1) * H],
                        lhsT=hsb[:, t * P:(t + 1) * P],
                        rhs=w2at[:],
                        start=True, stop=True, skip_group_check=True)
                msgt = kp.tile([P, CHUNK * H], bf, tag="msgt")
                nc.scalar.activation(msgt[:, :tpc * H],
                                     msgps[:, :tpc * H], AF.Silu)
                ctiles[k] = (ctiles[k][1], msgt)   # (selc, msgt)

            def emit_scatter(k):
                w, c0, tpc, _, _ = chunks[k]
                selc, msgt = ctiles.pop(k)
                t_w = int(tw[w])
                aggps = wpsum[w]
                for t in range(tpc):
                    tt = c0 + t
                    # agg[node, feat] += sel[e, n]^T @ msg[e, f]
                    nc.tensor.matmul(
                        aggps[:],
                        lhsT=selc[:, t * P:(t + 1) * P],
                        rhs=msgt[:, t * H:(t + 1) * H],
                        start=(tt == 0), stop=(tt == t_w - 1),
                        skip_group_check=True)

            def emit_tail(w):
                # node MLP (feat-major): out = silu(W3^T [x_win; agg] + b3)
                wtiles.pop(w)
                g = w // GRP
                dct8, nit8, oo8, _ = gtiles[g]
                aggps = wpsum.pop(w)
                a2sb = kp.tile([P, H], bf, tag="a2sb")
                nc.vector.tensor_copy(out=a2sb[:], in_=aggps[:])
                aggT = px.tile([H, P], bf, tag="aggT", name="aggT")
                nc.tensor.transpose(aggT[:], a2sb[:], identt[:])
                aggTsb = kp.tile([H, P], bf, tag="aggTsb")
                nc.vector.tensor_copy(out=aggTsb[:], in_=aggT[:])
                wo = (w - gw0[g]) * P
                ops = px.tile([H, P], f32, tag="ops", name="ops")
                nc.tensor.matmul(ops[:], lhsT=w3at[:],
                                 rhs=nit8[:, wo:wo + P],
                                 start=True, stop=False, skip_group_check=True)
                nc.tensor.matmul(ops[:], lhsT=w3bt[:], rhs=aggTsb[:],
                                 start=False, stop=True, skip_group_check=True)
                nc.scalar.activation(oo8[:, wo:wo + P], ops[:], AF.Silu,
                                     bias=b3t[:])
                if w == gw1[g] - 1:
                    nc.sync.dma_start(
                        out=outT[:, gw0[g] * P:gw1[g] * P],
                        in_=oo8[:, :(gw1[g] - gw0[g]) * P])
                    del gtiles[g]

            # software-pipelined emission: h-matmuls run one chunk ahead
            # of msg/scatter so PE stays busy during silu on Scalar.
            emit_group_dma(0)
            emit_dma(0)
            pending_tail = []
            for k, ch in enumerate(chunks):
                w, _, _, first, _ = ch
                if first and w + 1 < nw:
                    emit_dma(w + 1)
                if first and w % GRP == 0 and w // GRP + 1 < ngrp:
                    emit_group_dma(w // GRP + 1)
                emit_h1(k)
                if k > 0:
                    emit_msg(k - 1)
                emit_h(k)
                if k > 0:
                    emit_scatter(k - 1)
                if pending_tail:
                    emit_tail(pending_tail.pop(0))
                if k > 0 and chunks[k - 1][4]:
                    pending_tail.append(chunks[k - 1][0])
            emit_msg(len(chunks) - 1)
            emit_scatter(len(chunks) - 1)
            for w in pending_tail:
                emit_tail(w)
            emit_tail(chunks[-1][0])

    nc.compile()
    return nc


# ---------------------------------------------------------------- entry

def kernel(x, edge_index, edge_attr, W1, b1, W2, b2, W3, b3):
    import time
    t0 = time.time()
    x = np.asarray(x, dtype=np.float32)
    edge_index = np.asarray(edge_index)
    edge_attr = np.asarray(edge_attr, dtype=np.float32)

    struct, arrays = _prep(x, edge_index, edge_attr)
    consts = _prep_consts(
        np.asarray(W1, np.float32), np.asarray(b1, np.float32),
        np.asarray(W2, np.float32), np.asarray(b2, np.float32),
        np.asarray(W3, np.float32), np.asarray(b3, np.float32))
    t1 = time.time()

    nc = _build(struct)
    t2 = time.time()
    print(f"[kernel] prep {t1 - t0:.1f}s  build+tile {t2 - t1:.1f}s")

    from concourse.bass_utils import run_bass_kernel_spmd
    in_maps = []
    for c in range(N_CORES):
        m = {
            "xsA": arrays["xsA"][c], "xsB": arrays["xsB"][c],
            "dstc": arrays["dstc"][c], "xTn": arrays["xTn"][c],
        }
        m.update(consts)
        in_maps.append(m)
    t3 = time.time()
    res = run_bass_kernel_spmd(nc, in_maps, core_ids=list(range(N_CORES)))
    print(f"[kernel] compile+run {time.time() - t3:.1f}s")

    nw = struct["nw"]
    order_w = struct["order_w"]
    nwg_pad = struct["nwg_pad"]
    n_nodes = struct["n_nodes"]
    out_full = np.empty((nwg_pad * P, H), dtype=np.float32)
    colidx = np.arange(P)
    for c in range(N_CORES):
        gws = order_w[np.arange(nw) * N_CORES + c]
        idx = (gws[:, None] * P + colidx[None, :]).ravel()
        out_full[idx, :] = res.results[c]["outT"].T
    return out_full[:n_nodes]
